# revision 1
# baseline (speedup 1.0000x reference)
"""ArcFace (non-linear squashing) + cross-entropy loss, distributed over 8 TRN2 NeuronCores.

Strategy (classic model-parallel ArcFace head):
  - Host folds the per-row squashing scale into x:  xs = x * sqrt(||x||^2)/(||x||^2+1)
    and the per-class L2 normalization into w:      wn = w / ||w||_row
    so that cosine = xs @ wn.T  with |cosine| <= 1 (no logsumexp max-shift needed:
    exp(30*cos) <= e^30 fits fp32 comfortably).
  - Classes (50000) are sharded column-wise across 8 cores (6250 each). The small
    xs is replicated. Both are quantized and pre-transposed/interleaved so the
    contraction dim K=512 lands on SBUF partitions ([128, kc, *]: k = kc*128 + p).
  - Each core computes cosine tiles on the PE (fp32 PSUM accumulation; fp8
    DoubleRow packs 2 k-chunks per matmul), ScalarE does exp(30*cos) with a free
    per-partition running sum (accum_out), VectorE reduces the per-row max. Only
    [2,128,8] f32 leaves each core - the [1024, 50000] logits never touch HBM.
  - Host combines the 8 partial sums/maxes, applies the one-hot phi swap
    correction for the label column analytically, and forms (loss, acc).
    argmax(phi) == argmax(cosine) since phi is a strictly increasing function of
    cosine, so accuracy reduces to "is the label's cosine the row max".
"""

import math
import sys

import numpy as np

if "/opt/trn_rl_repo" not in sys.path:  # harmless if site config already provides it
    sys.path.insert(0, "/opt/trn_rl_repo")

import ml_dtypes

import concourse.bacc as bacc
import concourse.bass as bass
import concourse.mybir as mybir
from concourse import tile
from concourse.bass_utils import run_bass_kernel_spmd

# Problem constants (hardcoded per the harness contract)
B = 1024
K = 512
C = 50000
NCORES = 8
CSH = C // NCORES  # 6250 classes per core

M_MARGIN = 0.5
S = 30.0
COS_M = math.cos(M_MARGIN)
SIN_M = math.sin(M_MARGIN)
TH = math.cos(math.pi - M_MARGIN)
MM = math.sin(math.pi - M_MARGIN) * M_MARGIN

# ---- tunables (bench.py overrides these to explore variants) ----
MM_DTYPE = "fp8"     # "bf16" | "fp8"  (fp8 uses DoubleRow: 2 k-chunks per matmul)
MAX_SRC = "exp2"     # "psum": f32 cosine max | "exp2": max over bf16 exp tile in SBUF
CG = 2048            # classes per PSUM group tile (multiple of 512, <= 4096)
EX_BUFS = 4          # exp scratch buffer depth (hides the DVE max under PE/ACT)
W_BUFS = 3           # weight tile prefetch depth
TREE_MAX = True      # shrink 4x with TT-max (2x packed) before the 1x reduce

_NC_CACHE = {}


def class_groups(cg):
    groups = []
    c0 = 0
    while c0 < CSH:
        sz = min(cg, CSH - c0)
        groups.append((c0, sz))
        c0 += sz
    return groups


def build_nc(repeat=1, mm_dtype=None, max_src=None, cg=None, skip=()):
    """Build + compile the per-core Bass program (same graph on all 8 cores).

    repeat > 1 re-runs the whole body N times inside one NEFF (benchmarking
    only - lets slope timing cancel per-execution dispatch overhead).
    """
    mm_dtype = mm_dtype or MM_DTYPE
    max_src = max_src or MAX_SRC
    cg = cg or CG

    bf16 = mybir.dt.bfloat16
    f32 = mybir.dt.float32
    in_dt = mybir.dt.float8e4 if mm_dtype == "fp8" else bf16
    groups = class_groups(cg)
    ng = len(groups)
    nbank = cg // 512  # psum banks per group tile

    nc = bacc.Bacc(
        "TRN2",
        target_bir_lowering=False,
        debug=False,
        num_devices=NCORES,
    )

    xsT_d = nc.dram_tensor("xsT", [K, B], in_dt, kind="ExternalInput")
    wnT_d = nc.dram_tensor("wnT", [K, CSH], in_dt, kind="ExternalInput")
    out_d = nc.dram_tensor("out", [2, 128, 8], f32, kind="ExternalOutput")

    with tile.TileContext(nc) as tc:
        with (
            tc.tile_pool(name="xs", bufs=1) as xs_pool,
            tc.tile_pool(name="w", bufs=W_BUFS) as w_pool,
            tc.tile_pool(name="ps", bufs=8 // nbank, space=bass.MemorySpace.PSUM) as ps_pool,
            tc.tile_pool(name="ex", bufs=EX_BUFS) as ex_pool,
            tc.tile_pool(name="st", bufs=1) as st_pool,
        ):
            # xs resident in SBUF as [p, kc, b]: k = kc*128 + p
            xs_sb = xs_pool.tile([128, 4, B], in_dt, tag="xs")
            xsT_r = xsT_d.ap().rearrange("(kc p) b -> p kc b", p=128)
            nc.sync.dma_start(xs_sb[:], xsT_r)

            # per-(b_chunk, c_group) partial stats; column b*8 + gi
            sumbuf = st_pool.tile([128, 8 * ng], f32, tag="sumbuf")
            maxbuf = st_pool.tile([128, 8 * ng], f32, tag="maxbuf")
            sums = st_pool.tile([128, 8], f32, tag="sums")
            maxs = st_pool.tile([128, 8], f32, tag="maxs")
            maskend = None
            if max_src == "exp":
                # full-window mask end (any value > cg keeps every element)
                maskend = st_pool.tile([128, 1], f32, tag="maskend")
                nc.gpsimd.memset(maskend[:], float(cg + 1))

            # source view of wnT with partition inside: [p, kc, c]
            wnT_r = wnT_d.ap().rearrange("(kc p) c -> p kc c", p=128)

            for _rep in range(repeat):
                for gi, (c0, sz) in enumerate(groups):
                    w_t = w_pool.tile([128, 4, cg], in_dt, tag="w")
                    nc.sync.dma_start(w_t[:, :, :sz], wnT_r[:, :, c0 : c0 + sz])
                    nsub = (sz + 511) // 512
                    for b in range(8):
                        ps = ps_pool.tile([128, cg], f32, tag="ps")
                        if mm_dtype == "fp8":
                            # g outer / h inner: consecutive matmuls share the
                            # stationary operand (same b, g), easing LDWEIGHTS
                            # pressure; PSUM has_written bits handle the
                            # interleaved accumulation groups per h-slice.
                            for g in range(2):
                                for h in range(nsub):
                                    h0 = h * 512
                                    hsz = min(512, sz - h0)
                                    nc.tensor.matmul(
                                        ps[:, h0 : h0 + hsz],
                                        xs_sb[:, 2 * g : 2 * g + 2, b * 128 : b * 128 + 128],
                                        w_t[:, 2 * g : 2 * g + 2, h0 : h0 + hsz],
                                        start=(g == 0),
                                        stop=(g == 1),
                                        perf_mode=mybir.MatmulPerfMode.DoubleRow,
                                        skip_group_check=True,
                                    )
                        else:
                            for k in range(4):
                                for h in range(nsub):
                                    h0 = h * 512
                                    hsz = min(512, sz - h0)
                                    nc.tensor.matmul(
                                        ps[:, h0 : h0 + hsz],
                                        xs_sb[:, k, b * 128 : b * 128 + 128],
                                        w_t[:, k, h0 : h0 + hsz],
                                        start=(k == 0),
                                        stop=(k == 3),
                                        skip_group_check=True,
                                    )
                        ex = ex_pool.tile([128, cg], bf16, tag="ex")
                        if "act" not in skip:
                            nc.scalar.activation(
                                ex[:, :sz],
                                ps[:, :sz],
                                mybir.ActivationFunctionType.Exp,
                                scale=S,
                                accum_out=sumbuf[:, b * ng + gi : b * ng + gi + 1],
                            )
                        if "dve" in skip:
                            if "act" in skip:
                                # light PSUM consumer so matmuls aren't dead
                                # (reads only 8 columns -> exposes pure PE rate)
                                nc.scalar.activation(
                                    ex[:, :8],
                                    ps[:, :8],
                                    mybir.ActivationFunctionType.Copy,
                                    scale=1.0,
                                )
                        elif max_src == "exp":
                            # max over bf16 exp values (exp is monotone in cosine)
                            # via the native TENSOR_MASK_REDUCE opcode with a
                            # full-window mask.
                            ex2 = ex_pool.tile([128, cg], bf16, tag="ex2")
                            nc.vector.tensor_mask_reduce(
                                out=ex2[:, :sz],
                                in_=ex[:, :sz],
                                mask_start=0.0,
                                mask_end=maskend[:],
                                scale=1.0,
                                accum_in=0.0,
                                op=mybir.AluOpType.max,
                                accum_out=maxbuf[:, b * ng + gi : b * ng + gi + 1],
                            )
                        elif max_src == "exp2":
                            # free-axis max over the bf16 exp tile in SBUF.
                            # For full groups, shrink 4x first with two
                            # tensor_tensor max ops (2x-packed bf16 mode) so the
                            # 1x-rate tensor_reduce touches a quarter of the
                            # elements. max is exact in bf16 (picks an input).
                            mcol = maxbuf[:, b * ng + gi : b * ng + gi + 1]
                            if TREE_MAX and sz == cg and cg >= 2048:
                                t1 = ex_pool.tile([128, cg // 2], bf16, tag="mx1")
                                nc.vector.tensor_max(
                                    t1[:], ex[:, : cg // 2], ex[:, cg // 2 : cg]
                                )
                                t2 = ex_pool.tile([128, cg // 4], bf16, tag="mx2")
                                nc.vector.tensor_max(
                                    t2[:], t1[:, : cg // 4], t1[:, cg // 4 : cg // 2]
                                )
                                nc.vector.tensor_reduce(
                                    mcol,
                                    t2[:],
                                    axis=mybir.AxisListType.X,
                                    op=mybir.AluOpType.max,
                                )
                            else:
                                nc.vector.tensor_reduce(
                                    mcol,
                                    ex[:, :sz],
                                    axis=mybir.AxisListType.X,
                                    op=mybir.AluOpType.max,
                                )
                        else:
                            nc.vector.tensor_reduce(
                                maxbuf[:, b * ng + gi : b * ng + gi + 1],
                                ps[:, :sz],
                                axis=mybir.AxisListType.X,
                                op=mybir.AluOpType.max,
                            )

            for b in range(8):
                if "act" not in skip:
                    nc.vector.tensor_reduce(
                        sums[:, b : b + 1],
                        sumbuf[:, b * ng : b * ng + ng],
                        axis=mybir.AxisListType.X,
                        op=mybir.AluOpType.add,
                    )
                if "dve" not in skip:
                    nc.vector.tensor_reduce(
                        maxs[:, b : b + 1],
                        maxbuf[:, b * ng : b * ng + ng],
                        axis=mybir.AxisListType.X,
                        op=mybir.AluOpType.max,
                    )

            out_ap = out_d.ap()
            if "act" in skip:
                nc.gpsimd.memset(sums[:], 0.0)
            if "dve" in skip:
                nc.gpsimd.memset(maxs[:], 0.0)
            nc.sync.dma_start(out_ap[0], sums[:])
            nc.sync.dma_start(out_ap[1], maxs[:])

    nc.compile()
    return nc


def get_nc(repeat=1, mm_dtype=None, max_src=None, cg=None, skip=()):
    key = (repeat, mm_dtype or MM_DTYPE, max_src or MAX_SRC, cg or CG, tuple(skip),
           EX_BUFS, W_BUFS, TREE_MAX)
    if key not in _NC_CACHE:
        _NC_CACHE[key] = build_nc(repeat, mm_dtype, max_src, cg, skip)
    return _NC_CACHE[key]


def quantize_host(x, w, mm_dtype=None):
    """Host prep: fold squashing scale into x, L2 norm into w; quantize;
    lay out as [K, B] / [K, C] with K rows (k = kc*128 + p after rearrange)."""
    mm_dtype = mm_dtype or MM_DTYPE
    qdt = ml_dtypes.float8_e4m3 if mm_dtype == "fp8" else ml_dtypes.bfloat16
    sq = np.einsum("bk,bk->b", x, x)
    xs = x * (np.sqrt(sq) / (sq + 1.0))[:, None]
    wn = w / np.sqrt(np.einsum("ck,ck->c", w, w))[:, None]
    xs_q = xs.astype(qdt)
    wn_q = wn.astype(qdt)
    xsT = np.ascontiguousarray(xs_q.T)  # [K, B]
    wnT = np.ascontiguousarray(wn_q.T)  # [K, C]
    return xs_q, wn_q, xsT, wnT


def kernel(input, label, weight):
    x = np.asarray(input, dtype=np.float64)  # [B, K]
    lab = np.asarray(label).astype(np.int64)  # [B]
    w = np.asarray(weight, dtype=np.float64)  # [C, K]

    xs_q, wn_q, xsT, wnT = quantize_host(x, w)

    in_maps = [
        {"xsT": xsT, "wnT": np.ascontiguousarray(wnT[:, i * CSH : (i + 1) * CSH])}
        for i in range(NCORES)
    ]

    nc = get_nc()
    results = run_bass_kernel_spmd(nc, in_maps, core_ids=list(range(NCORES))).results

    # combine per-core partials: out[0][p, b] = sum_exp for row b*128+p, out[1] max
    SE = np.zeros(B, dtype=np.float64)
    MX = np.full(B, -np.inf)
    for r in results:
        o = np.asarray(r["out"], dtype=np.float64)  # [2, 128, 8]
        SE += o[0].T.reshape(B)
        MX = np.maximum(MX, o[1].T.reshape(B))

    # label-column correction on host, with the same quantized values the device saw
    xs_f = xs_q.astype(np.float64)
    wn_f = wn_q.astype(np.float64)
    coslab = np.einsum("bk,bk->b", xs_f, wn_f[lab])
    sine = np.sqrt(np.clip(1.0 - coslab * coslab, 0.0, 1.0))
    phi = np.where(coslab > TH, coslab * COS_M - sine * SIN_M, coslab - MM)

    total = SE - np.exp(S * coslab) + np.exp(S * phi)
    loss = np.mean(np.log(total) - S * phi)
    if MAX_SRC in ("exp", "exp2"):
        # MX is in bf16-rounded exp domain: exp(S*cos) rel step ~2^-8
        acc = 100.0 * np.mean(np.exp(S * coslab) >= MX * (1.0 - 0.008))
    else:
        acc = 100.0 * np.mean(coslab >= MX - 2e-6)

    return (np.float32(loss), np.float32(acc))

